# revision 3
# baseline (speedup 1.0000x reference)
"""Trainium2 Bass kernel for nn_Attention_51376398794919.

Dense transformer block: LayerNorm -> QKV -> attention with relative-position
bias -> proj.  Data-parallel over batch across 8 NeuronCores (4 batches/core).

Device-side layout strategy (per core):
  - LN in natural layout [tok, d]; 1/sigma via exp(-0.5*ln(var+eps)) on the
    ACT engine (ln+exp live in one activation-table set -> no table swaps);
    xn transposed to xnT [d, tok] via PE transposes (stored bf16).
  - qkT (q/k head-transposed, [d_head, tok]) computed from xnT and quantized
    to fp8e4m3 (scaled x2 on the host); scores run as a single fp8 DoubleRow
    matmul per (k-tile, chunk) with a stride-0 pair dim (reads each operand
    twice -> 2x result, folded into the exp scale 1/64).
  - Relative-position bias applied multiplicatively: host precomputes
    expb = exp(bias^T) bf16; est = exp(scores/64) (ACT) then est *= expb
    (DVE tensor_tensor).  No bias identity-matmuls on the PE.
  - PV: out[q, d] = expST.T @ [v | ones]; the ones column yields the softmax
    denominator; normalization on ACT (Copy with per-partition 1/den scale).
  - attn transposed back (PE) into an SBUF slab read by proj.
Software pipelining: qkv+scores+exp run one head ahead of PV; the attn-out
transposes/copies of head h are deferred into head h+1's window (they wait on
the ACT an-chain); LN of batch b+1 is emitted during heads 4-6 of batch b and
qkv/scores of (head 0, b+1) before pv(7, b)/proj(b).  The PE never idles.
"""

import sys

import numpy as np

sys.path.insert(0, "/opt/trn_rl_repo")

import concourse.bacc as bacc
import concourse.mybir as mybir
import concourse.tile as tile
from concourse.bass_utils import run_bass_kernel_spmd

# Problem constants
B, N, DIM = 32, 1024, 512
H, KD, D = 8, 64, 256
DH = D * H  # 2048
SCALE = KD ** -0.5
NCORES = 8
BL = B // NCORES  # 4 batches per core

F32 = mybir.dt.float32
BF16 = mybir.dt.bfloat16
E4 = mybir.dt.float8e4
AF = mybir.ActivationFunctionType
ALU = mybir.AluOpType
DR = mybir.MatmulPerfMode.DoubleRow

KT = N // 128    # 8 k-tiles
QS = N // 128    # 8 q-slices
DT = DIM // 128  # 4 d-tiles
VW = 257         # v-hat width: 256 v + 1 ones
QK_PRE = 2.0     # host pre-scale on q and k before fp8 quantization
# scores psum = 2 (stride-0 pair) * (2q)*(2k) = 8*qk ; exp scale recovers /8
EXP_SCALE = 1.0 / (2 * QK_PRE * QK_PRE) * SCALE


def dup2(ap):
    """Insert a stride-0 count-2 dim after the partition dim (DoubleRow
    pair that reads the same block twice -> result is 2x)."""
    new = ap.copy()
    new.ap = [ap.ap[0]] + [[0, 2]] + ap.ap[1:]
    return new


def build_program(use_qk_bias=False, use_v_bias=False, use_pb=False):
    nc = bacc.Bacc("TRN2", target_bir_lowering=False, debug=True)

    x_d = nc.declare_dram_parameter("x", [BL, N, DIM], F32, isOutput=False)
    wqk_d = nc.declare_dram_parameter("wqk", [DIM, H * 128], BF16, isOutput=False)
    wv_d = nc.declare_dram_parameter("wv", [DIM, DH], BF16, isOutput=False)
    bqk_d = nc.declare_dram_parameter("bqk", [1, H * 128], BF16, isOutput=False)
    bv1_d = nc.declare_dram_parameter("bv1", [1, H * VW], BF16, isOutput=False)
    pw_d = nc.declare_dram_parameter("pw", [DH, DIM], BF16, isOutput=False)
    pb1_d = nc.declare_dram_parameter("pb1", [1, DIM], BF16, isOutput=False)
    expb_d = nc.declare_dram_parameter("expb", [H, N, N], BF16, isOutput=False)
    identb_d = nc.declare_dram_parameter("identb", [128, 128], BF16, isOutput=False)
    ones_d = nc.declare_dram_parameter("ones", [1, 512], BF16, isOutput=False)
    y_d = nc.declare_dram_parameter("y", [BL, N, DIM], F32, isOutput=True)

    with tile.TileContext(nc) as tc:
        with (
            tc.tile_pool(name="consts", bufs=1) as cpool,
            tc.tile_pool(name="xnt", bufs=2) as xpool,
            tc.tile_pool(name="slab", bufs=1) as slabpool,
            tc.tile_pool(name="yout", bufs=3) as ypool,
            tc.tile_pool(name="ln", bufs=4) as lpool,
            tc.tile_pool(name="stats", bufs=16) as spool,
            tc.tile_pool(name="bias", bufs=16) as bpool,
            tc.tile_pool(name="qk", bufs=3) as qkpool,
            tc.tile_pool(name="vhat", bufs=3) as vpool,
            tc.tile_pool(name="expst", bufs=16) as epool,
            tc.tile_pool(name="attn", bufs=16) as apool,
            tc.tile_pool(name="stp", bufs=3, space="PSUM") as stpp,
            tc.tile_pool(name="pvp", bufs=3, space="PSUM") as pvpp,
            tc.tile_pool(name="miscp", bufs=2, space="PSUM") as mpp,
        ):
            # ---- constants; identb first so LN transposes can start early
            identb = cpool.tile([128, 128], BF16)
            nc.sync.dma_start(identb[:], identb_d[:])
            eps_t = cpool.tile([128, 1], F32)
            nc.vector.memset(eps_t[:], 1e-5)

            def load_consts():
                if use_qk_bias or use_v_bias or use_pb:
                    ones_bf = cpool.tile([1, 512], BF16)
                    nc.sync.dma_start(ones_bf[:], ones_d[:])
                else:
                    ones_bf = None
                wqk = cpool.tile([128, DT * H * 128], BF16)
                for dt in range(DT):
                    for hh in range(2):
                        nc.sync.dma_start(
                            wqk[:, dt * H * 128 + hh * 512:
                                dt * H * 128 + (hh + 1) * 512],
                            wqk_d[dt * 128:(dt + 1) * 128,
                                  hh * 512:(hh + 1) * 512],
                        )
                if use_qk_bias:
                    bqk = cpool.tile([1, H * 128], BF16)
                    nc.sync.dma_start(bqk[:], bqk_d[:])
                else:
                    bqk = None
                wv = cpool.tile([128, DT * DH], BF16)
                for dt in range(DT):
                    for hh in range(2):
                        nc.sync.dma_start(
                            wv[:, dt * DH + hh * (DH // 2):
                               dt * DH + (hh + 1) * (DH // 2)],
                            wv_d[dt * 128:(dt + 1) * 128,
                                 hh * (DH // 2):(hh + 1) * (DH // 2)],
                        )
                if use_v_bias:
                    bv1 = cpool.tile([1, H * VW], BF16)
                    nc.sync.dma_start(bv1[:], bv1_d[:])
                else:
                    bv1 = None
                pw = cpool.tile([128, 16 * DIM], BF16)
                for dh in range(16):
                    nc.sync.dma_start(
                        pw[:, dh * DIM:(dh + 1) * DIM],
                        pw_d[dh * 128:(dh + 1) * 128, :],
                    )
                if use_pb:
                    pb1 = cpool.tile([1, DIM], BF16)
                    nc.sync.dma_start(pb1[:], pb1_d[:])
                else:
                    pb1 = None
                return ones_bf, wqk, bqk, wv, bv1, pw, pb1

            consts = [None]

            def emit_ln(b, sls, xnt):
                """LayerNorm slices `sls` of batch b into xnT [d, tok].
                1/sigma = exp(-0.5*ln(var+eps)) keeps ACT on one table set."""
                for sl in sls:
                    xt = lpool.tile([128, DIM], F32, tag="x")
                    nc.sync.dma_start(xt[:], x_d[b, sl * 128:(sl + 1) * 128, :])
                    st6 = spool.tile([128, 6], F32, tag="st6")
                    nc.vector.bn_stats(st6[:], xt[:])
                    mv = spool.tile([128, 2], F32, tag="mv")
                    nc.vector.bn_aggr(mv[:], st6[:])
                    lv = spool.tile([128, 1], F32, tag="lv")
                    nc.scalar.activation(lv[:], mv[:, 1:2], AF.Ln, bias=eps_t[:])
                    rs = spool.tile([128, 1], F32, tag="rs")
                    nc.scalar.activation(rs[:], lv[:], AF.Exp, scale=-0.5)
                    nm = spool.tile([128, 1], F32, tag="nm")
                    nc.vector.tensor_scalar(
                        nm[:], mv[:, 0:1], rs[:], -1.0, ALU.mult, ALU.mult
                    )
                    xn = lpool.tile([128, DIM], BF16, tag="xn")
                    nc.vector.tensor_scalar(
                        xn[:], xt[:], rs[:], nm[:], ALU.mult, ALU.add
                    )
                    for dt in range(DT):
                        tp = mpp.tile([128, 128], BF16, tag="m")
                        nc.tensor.transpose(
                            tp[:], xn[:, dt * 128:(dt + 1) * 128], identb[:]
                        )
                        nc.vector.tensor_copy(
                            xnt[:, dt * N + sl * 128: dt * N + (sl + 1) * 128],
                            tp[:],
                        )

            def emit_qkv(h, xnt):
                """expb tiles, qT/kT (fp8), v-hat for head h."""
                ones_bf, wqk, bqk, wv, bv1, pw, pb1 = consts[0]
                btiles = []
                for kt in range(KT):
                    bt = bpool.tile([128, N], BF16, tag="bias")
                    nc.sync.dma_start(
                        bt[:], expb_d[h, kt * 128:(kt + 1) * 128, :]
                    )
                    btiles.append(bt)
                qt = qkpool.tile([64, N], E4, tag="qt")
                ktile = qkpool.tile([64, N], E4, tag="kt")
                for c in range(2):
                    qp = mpp.tile([128, 512], F32, tag="m")
                    for dt in range(DT):
                        nc.tensor.matmul(
                            qp[:],
                            wqk[:, dt * H * 128 + h * 128:
                                dt * H * 128 + (h + 1) * 128],
                            xnt[:, dt * N + c * 512: dt * N + (c + 1) * 512],
                            start=(dt == 0),
                            stop=(not use_qk_bias and dt == DT - 1),
                        )
                    if use_qk_bias:
                        nc.tensor.matmul(
                            qp[:],
                            bqk[:, h * 128:(h + 1) * 128],
                            ones_bf[:, 0:512],
                            start=False,
                            stop=True,
                        )
                    nc.vector.tensor_copy(
                        qt[:, c * 512:(c + 1) * 512], qp[0:64, :]
                    )
                    nc.vector.tensor_copy(
                        ktile[:, c * 512:(c + 1) * 512], qp[64:128, :]
                    )
                vh = vpool.tile([128, KT * VW], BF16, tag="vh")
                nc.vector.memset(
                    vh[:].rearrange("p (s w) -> p s w", w=VW)[:, :, 256:257],
                    1.0,
                )
                for sl in range(QS):
                    vp = pvpp.tile([128, VW], F32, tag="pv")
                    for dt in range(DT):
                        nc.tensor.matmul(
                            vp[:, 0:256],
                            xnt[:, dt * N + sl * 128: dt * N + (sl + 1) * 128],
                            wv[:, dt * DH + h * 256: dt * DH + (h + 1) * 256],
                            start=(dt == 0),
                            stop=(not use_v_bias and dt == DT - 1),
                        )
                    if use_v_bias:
                        nc.tensor.matmul(
                            vp[:, 0:256],
                            ones_bf[:, 0:128],
                            bv1[:, h * VW: h * VW + 256],
                            start=False,
                            stop=True,
                            skip_group_check=True,
                        )
                    nc.vector.tensor_copy(
                        vh[:, sl * VW: sl * VW + 256], vp[:, 0:256]
                    )
                return btiles, qt, ktile, vh

            def emit_st(hctx):
                """Scores (fp8 DoubleRow) -> exp -> *= expb; transposed
                layout est[kt][k, q]."""
                btiles, qt, ktile, vh = hctx
                est = []
                for kt in range(KT):
                    et = epool.tile([128, N], BF16, tag="e")
                    for c in range(2):
                        cs = slice(c * 512, (c + 1) * 512)
                        sp = stpp.tile([128, 512], F32, tag="st")
                        nc.tensor.matmul(
                            sp[:],
                            dup2(ktile[:, kt * 128:(kt + 1) * 128]),
                            dup2(qt[:, cs]),
                            start=True, stop=True, perf_mode=DR,
                        )
                        nc.scalar.activation(
                            et[:, cs], sp[:], AF.Exp, bias=0.0, scale=EXP_SCALE
                        )
                        nc.vector.tensor_tensor(
                            et[:, cs], et[:, cs], btiles[kt][:, cs], ALU.mult
                        )
                    est.append(et)
                return est

            def emit_pv_mm(hctx, est):
                """PV matmuls + denominator reciprocal + normalized attn
                (ACT Copy with per-partition scale).  Returns an-tiles."""
                btiles, qt, ktile, vh = hctx
                ans = []
                for sl in range(QS):
                    pv = pvpp.tile([128, VW], F32, tag="pv")
                    for kt in range(KT):
                        nc.tensor.matmul(
                            pv[:],
                            est[kt][:, sl * 128:(sl + 1) * 128],
                            vh[:, kt * VW:(kt + 1) * VW],
                            start=(kt == 0),
                            stop=(kt == KT - 1),
                        )
                    rc = spool.tile([128, 1], F32, tag="rc")
                    nc.vector.reciprocal(rc[:], pv[:, 256:257])
                    an = apool.tile([128, 256], BF16, tag="an")
                    nc.scalar.mul(an[:], pv[:, 0:256], rc[:])
                    ans.append(an)
                return ans

            def emit_pv_fin(h, ans, slab):
                """Deferred: transpose normalized attn into the proj slab."""
                for sl in range(QS):
                    for dt in range(2):
                        tp = mpp.tile([128, 128], BF16, tag="m")
                        nc.tensor.transpose(
                            tp[:], ans[sl][:, dt * 128:(dt + 1) * 128],
                            identb[:]
                        )
                        nc.vector.tensor_copy(
                            slab[:, (h * 2 + dt) * N + sl * 128:
                                 (h * 2 + dt) * N + (sl + 1) * 128],
                            tp[:],
                        )

            def emit_proj(b, slab):
                ones_bf, wqk, bqk, wv, bv1, pw, pb1 = consts[0]
                for sl in range(QS):
                    pp = stpp.tile([128, DIM], F32, tag="st")
                    for dh in range(16):
                        nc.tensor.matmul(
                            pp[:],
                            slab[:, dh * N + sl * 128: dh * N + (sl + 1) * 128],
                            pw[:, dh * DIM:(dh + 1) * DIM],
                            start=(dh == 0),
                            stop=(not use_pb and dh == 15),
                        )
                    if use_pb:
                        nc.tensor.matmul(
                            pp[:], ones_bf[:, 0:128], pb1[:], start=False,
                            stop=True, skip_group_check=True,
                        )
                    yt = ypool.tile([128, DIM], F32, tag="y")
                    nc.vector.tensor_copy(yt[:], pp[:])
                    nc.sync.dma_start(y_d[b, sl * 128:(sl + 1) * 128, :], yt[:])

            # ---- software-pipelined main loop --------------------------
            LN_AT = {4: range(0, 3), 5: range(3, 6), 6: range(6, 8)}
            xnt_cur = xpool.tile([128, DT * N], BF16, tag="xnt")
            emit_ln(0, range(QS), xnt_cur)
            consts[0] = load_consts()
            hctx_cur = emit_qkv(0, xnt_cur)
            est_cur = emit_st(hctx_cur)
            slab = slabpool.tile([128, 16 * N], BF16, tag="slab")
            xnt_next = None
            pending = None
            for b in range(BL):
                for h in range(H):
                    if h + 1 < H:
                        if b + 1 < BL and h in LN_AT:
                            if h == 4:
                                xnt_next = xpool.tile(
                                    [128, DT * N], BF16, tag="xnt", name="xnt2"
                                )
                            emit_ln(b + 1, LN_AT[h], xnt_next)
                        hctx_nxt = emit_qkv(h + 1, xnt_cur)
                        est_nxt = emit_st(hctx_nxt)
                    elif b + 1 < BL:
                        hctx_nxt = emit_qkv(0, xnt_next)
                        est_nxt = emit_st(hctx_nxt)
                    else:
                        hctx_nxt = est_nxt = None
                    if pending is not None:
                        emit_pv_fin(*pending, slab)
                        pending = None
                    ans = emit_pv_mm(hctx_cur, est_cur)
                    pending = (h, ans)
                    hctx_cur, est_cur = hctx_nxt, est_nxt
                emit_pv_fin(*pending, slab)
                pending = None
                emit_proj(b, slab)
                xnt_cur = xnt_next

    nc.compile()
    return nc


_CACHE = {}


def _prep_host(gamma, beta, qkv_w, qkv_b, proj_w, proj_b, biases, bias_idxs):
    import ml_dtypes

    qkv_w = np.asarray(qkv_w, np.float32)
    qkv_b = np.asarray(qkv_b, np.float32)
    gamma = np.asarray(gamma, np.float32)
    beta = np.asarray(beta, np.float32)
    w = qkv_w * gamma[:, None]          # fold LN gamma
    bfold = qkv_b + beta @ qkv_w        # fold LN beta
    w3 = w.reshape(DIM, H, 384)
    b3 = bfold.reshape(H, 384)
    # q/k columns scaled x2 for fp8 range; exp scale divides it back out
    wqk = (w3[:, :, :128] * QK_PRE).reshape(DIM, H * 128)
    bqk = (b3[:, :128] * QK_PRE).reshape(1, H * 128)
    wv = w3[:, :, 128:].reshape(DIM, DH)
    bv = b3[:, 128:]                    # [H, 256]
    bv1 = np.concatenate(
        [bv, np.ones((H, 1), np.float32)], axis=1,
    ).reshape(1, H * VW)
    bias_full = np.asarray(biases, np.float32)[:, np.asarray(bias_idxs)]
    # device multiplies est[k, q] by exp(bias[q, k])^T
    expb = np.exp(bias_full.transpose(0, 2, 1))
    return {
        "wqk": wqk.astype(ml_dtypes.bfloat16),
        "wv": wv.astype(ml_dtypes.bfloat16),
        "bqk": bqk.astype(ml_dtypes.bfloat16),
        "bv1": bv1.astype(ml_dtypes.bfloat16),
        "pw": np.ascontiguousarray(np.asarray(proj_w, np.float32)).astype(ml_dtypes.bfloat16),
        "pb1": np.asarray(proj_b, np.float32).reshape(1, DIM).astype(ml_dtypes.bfloat16),
        "expb": np.ascontiguousarray(expb).astype(ml_dtypes.bfloat16),
        "identb": np.eye(128, dtype=np.float32).astype(ml_dtypes.bfloat16),
        "ones": np.ones((1, 512), ml_dtypes.bfloat16),
    }


def kernel(x, gamma, beta, qkv_w, qkv_b, proj_w, proj_b, biases, bias_idxs,
           _trace=False, _tmpdir=None):
    x = np.asarray(x, np.float32)
    shared = _prep_host(gamma, beta, qkv_w, qkv_b, proj_w, proj_b, biases,
                        bias_idxs)
    flags = (
        bool(np.any(np.asarray(shared["bqk"], np.float32))),
        bool(np.any(np.asarray(shared["bv1"], np.float32)
                    .reshape(H, VW)[:, :256])),
        bool(np.any(np.asarray(shared["pb1"], np.float32))),
    )
    if _CACHE.get("flags") != flags:
        _CACHE["nc"] = build_program(*flags)
        _CACHE["flags"] = flags
    nc = _CACHE["nc"]
    in_maps = []
    for c in range(NCORES):
        m = dict(shared)
        m["x"] = np.ascontiguousarray(x[c * BL:(c + 1) * BL])
        in_maps.append(m)
    res = run_bass_kernel_spmd(
        nc, in_maps, list(range(NCORES)), trace=_trace, tmpdir=_tmpdir,
    )
    _CACHE["last"] = res
    out = np.concatenate([res.results[c]["y"] for c in range(NCORES)], axis=0)
    return out.astype(np.float32)


# revision 6
# speedup vs baseline: 1.2382x; 1.2382x over previous
"""Trainium2 Bass kernel for nn_Attention_51376398794919.

Dense transformer block: LayerNorm -> QKV -> attention with relative-position
bias -> proj.  Data-parallel over batch across 8 NeuronCores (4 batches/core).

Device-side layout strategy (per core):
  - LN in natural layout [tok, d]; 1/sigma via a fast-inverse-sqrt bit trick
    + 2 Newton steps on the DVE (keeps the ACT engine on a single activation
    table set: only Exp and Copy are ever used -> one table load total);
    xn transposed to xnT [d, tok] via paired PE transposes (stored bf16).
  - qkT (q/k head-transposed, [d_head, tok]) computed from xnT and quantized
    to fp8e4m3 (scaled x2 on the host); scores run as a single fp8 DoubleRow
    matmul per (k-tile, chunk) with a stride-0 pair dim (reads each operand
    twice -> 2x result, folded into the exp scale 1/64).
  - Relative-position bias applied multiplicatively: host precomputes
    expb = exp(bias^T) bf16; est = exp(scores/64) (ACT) then est *= expb
    (DVE for k-tiles 0-4, GPSIMD for 5-7).  No bias matmuls on the PE.
  - PV: out[q, d] = expST.T @ [v | ones]; the ones column yields the softmax
    denominator; normalization on ACT (Copy with per-partition 1/den scale).
  - v-hat generated two token-slices per PSUM bank (shared-bank accumulation
    groups) so each PSUM->SBUF copy moves 512 columns; attn-out transposes
    are paired the same way.
Software pipelining: each head window interleaves, at k-tile granularity,
scores+exp+mult of head h+1, PV matmuls + normalization of head h, and the
deferred attn-out transposes/copies of head h-1 (which wait on the ACT
chain).  LN of batch b+1 is emitted during heads 4-6 of batch b; qkv/scores
of (head 0, b+1) run before pv(7, b); fin(7) interleaves into proj(b).
"""

import sys

import numpy as np

sys.path.insert(0, "/opt/trn_rl_repo")

import concourse.bacc as bacc
import concourse.mybir as mybir
import concourse.tile as tile
from concourse.bass_utils import run_bass_kernel_spmd

# Problem constants
B, N, DIM = 32, 1024, 512
H, KD, D = 8, 64, 256
DH = D * H  # 2048
SCALE = KD ** -0.5
NCORES = 8
BL = B // NCORES  # 4 batches per core

F32 = mybir.dt.float32
I32 = mybir.dt.int32
BF16 = mybir.dt.bfloat16
E4 = mybir.dt.float8e4
AF = mybir.ActivationFunctionType
ALU = mybir.AluOpType
DR = mybir.MatmulPerfMode.DoubleRow

KT = N // 128    # 8 k-tiles
QS = N // 128    # 8 q-slices
DT = DIM // 128  # 4 d-tiles
VW = 257         # v-hat width: 256 v + 1 ones
QK_PRE = 2.0     # host pre-scale on q and k before fp8 quantization
# scores psum = 2 (stride-0 pair) * (2q)*(2k) = 8*qk ; exp scale recovers /8
EXP_SCALE = 1.0 / (2 * QK_PRE * QK_PRE) * SCALE
POOL_MULT_KT = (5, 6, 7)  # est *= expb k-tiles run on GPSIMD


def dup2(ap):
    """Insert a stride-0 count-2 dim after the partition dim (DoubleRow
    pair that reads the same block twice -> result is 2x)."""
    new = ap.copy()
    new.ap = [ap.ap[0]] + [[0, 2]] + ap.ap[1:]
    return new


def build_program(use_qk_bias=False, use_v_bias=False, use_pb=False):
    nc = bacc.Bacc("TRN2", target_bir_lowering=False, debug=True)

    x_d = nc.declare_dram_parameter("x", [BL, N, DIM], F32, isOutput=False)
    wqk_d = nc.declare_dram_parameter("wqk", [DIM, H * 128], BF16, isOutput=False)
    wv_d = nc.declare_dram_parameter("wv", [DIM, DH], BF16, isOutput=False)
    bqk_d = nc.declare_dram_parameter("bqk", [1, H * 128], BF16, isOutput=False)
    bv1_d = nc.declare_dram_parameter("bv1", [1, H * VW], BF16, isOutput=False)
    pw_d = nc.declare_dram_parameter("pw", [DH, DIM], BF16, isOutput=False)
    pb1_d = nc.declare_dram_parameter("pb1", [1, DIM], BF16, isOutput=False)
    expb_d = nc.declare_dram_parameter("expb", [H, N, N], BF16, isOutput=False)
    identb_d = nc.declare_dram_parameter("identb", [128, 128], BF16, isOutput=False)
    ones_d = nc.declare_dram_parameter("ones", [1, 512], BF16, isOutput=False)
    y_d = nc.declare_dram_parameter("y", [BL, N, DIM], F32, isOutput=True)

    with tile.TileContext(nc) as tc:
        with (
            tc.tile_pool(name="consts", bufs=1) as cpool,
            tc.tile_pool(name="xnt", bufs=2) as xpool,
            tc.tile_pool(name="slab", bufs=1) as slabpool,
            tc.tile_pool(name="yout", bufs=3) as ypool,
            tc.tile_pool(name="lnx", bufs=8) as lxpool,
            tc.tile_pool(name="ln", bufs=3) as lpool,
            tc.tile_pool(name="stats", bufs=16) as spool,
            tc.tile_pool(name="bias", bufs=2) as bpool,
            tc.tile_pool(name="qk", bufs=3) as qkpool,
            tc.tile_pool(name="vhat", bufs=2) as vpool,
            tc.tile_pool(name="expst", bufs=16) as epool,
            tc.tile_pool(name="attn", bufs=10) as apool,
            tc.tile_pool(name="stp", bufs=3, space="PSUM") as stpp,
            tc.tile_pool(name="pvp", bufs=3, space="PSUM") as pvpp,
            tc.tile_pool(name="miscp", bufs=2, space="PSUM") as mpp,
        ):
            # ---- constants; identb first so LN transposes can start early
            identb = cpool.tile([128, 128], BF16)
            nc.sync.dma_start(identb[:], identb_d[:])
            eps_t = cpool.tile([128, 1], F32)
            nc.vector.memset(eps_t[:], 1e-5)

            def load_consts():
                if use_qk_bias or use_v_bias or use_pb:
                    ones_bf = cpool.tile([1, 512], BF16)
                    nc.sync.dma_start(ones_bf[:], ones_d[:])
                else:
                    ones_bf = None
                wqk = cpool.tile([128, DT * H * 128], BF16)
                for dt in range(DT):
                    for hh in range(2):
                        nc.sync.dma_start(
                            wqk[:, dt * H * 128 + hh * 512:
                                dt * H * 128 + (hh + 1) * 512],
                            wqk_d[dt * 128:(dt + 1) * 128,
                                  hh * 512:(hh + 1) * 512],
                        )
                if use_qk_bias:
                    bqk = cpool.tile([1, H * 128], BF16)
                    nc.sync.dma_start(bqk[:], bqk_d[:])
                else:
                    bqk = None
                wv = cpool.tile([128, DT * DH], BF16)
                for dt in range(DT):
                    for hh in range(2):
                        nc.sync.dma_start(
                            wv[:, dt * DH + hh * (DH // 2):
                               dt * DH + (hh + 1) * (DH // 2)],
                            wv_d[dt * 128:(dt + 1) * 128,
                                 hh * (DH // 2):(hh + 1) * (DH // 2)],
                        )
                if use_v_bias:
                    bv1 = cpool.tile([1, H * VW], BF16)
                    nc.sync.dma_start(bv1[:], bv1_d[:])
                else:
                    bv1 = None
                pw = cpool.tile([128, 16 * DIM], BF16)
                for dh in range(16):
                    nc.sync.dma_start(
                        pw[:, dh * DIM:(dh + 1) * DIM],
                        pw_d[dh * 128:(dh + 1) * 128, :],
                    )
                if use_pb:
                    pb1 = cpool.tile([1, DIM], BF16)
                    nc.sync.dma_start(pb1[:], pb1_d[:])
                else:
                    pb1 = None
                return ones_bf, wqk, bqk, wv, bv1, pw, pb1

            consts = [None]

            def emit_ln(b, sls, xnt):
                """LayerNorm slices `sls` of batch b into xnT [d, tok].
                1/sigma via bit-trick + 2 Newton steps, all on DVE."""
                L = len(sls)
                mvg = spool.tile([128, 2 * L], F32, tag="mvg")
                xts = []
                for j, sl in enumerate(sls):
                    xt = lxpool.tile([128, DIM], F32, tag="x")
                    nc.sync.dma_start(xt[:], x_d[b, sl * 128:(sl + 1) * 128, :])
                    xts.append(xt)
                    st6 = spool.tile([128, 6], F32, tag="st6")
                    nc.vector.bn_stats(st6[:], xt[:])
                    nc.vector.bn_aggr(mvg[:, 2 * j:2 * j + 2], st6[:])
                var_ap = mvg[:].rearrange("p (l two) -> p l two", two=2)[:, :, 1]
                ve = spool.tile([128, L], F32, tag="ve")
                nc.vector.tensor_scalar(ve[:], var_ap, eps_t[:], None, ALU.add)
                ti = spool.tile([128, L], I32, tag="ti")
                nc.vector.tensor_scalar(
                    ti[:], ve[:].bitcast(I32), 1, 0xFFFFFFFF,
                    ALU.logical_shift_right, ALU.bitwise_xor,
                )
                yi = spool.tile([128, L], I32, tag="yi")
                nc.vector.tensor_scalar(yi[:], ti[:], 0x5F3759E0, None, ALU.add)
                y = yi[:].bitcast(F32)
                aa = spool.tile([128, L], F32, tag="aa")
                cc = spool.tile([128, L], F32, tag="cc")
                for _ in range(2):
                    nc.vector.tensor_tensor(aa[:], y, y, ALU.mult)
                    nc.vector.tensor_tensor(aa[:], aa[:], ve[:], ALU.mult)
                    nc.vector.tensor_scalar(
                        cc[:], aa[:], -0.5, 1.5, ALU.mult, ALU.add
                    )
                    nc.vector.tensor_tensor(y, y, cc[:], ALU.mult)
                for j, sl in enumerate(sls):
                    rs = yi[:, j:j + 1].bitcast(F32)
                    nm = spool.tile([128, 1], F32, tag="nm")
                    nc.vector.tensor_scalar(
                        nm[:], mvg[:, 2 * j:2 * j + 1], rs, -1.0,
                        ALU.mult, ALU.mult
                    )
                    xn = lpool.tile([128, DIM], BF16, tag="xn")
                    nc.vector.tensor_scalar(
                        xn[:], xts[j][:], rs, nm[:], ALU.mult, ALU.add
                    )
                    for dp in range(2):  # pairs of d-tiles
                        tp = mpp.tile([128, 256], BF16, tag="m")
                        for e in range(2):
                            nc.tensor.matmul(
                                tp[:, e * 128:(e + 1) * 128],
                                xn[:, (2 * dp + e) * 128:(2 * dp + e + 1) * 128],
                                identb[:], is_transpose=True,
                                start=(e == 0), stop=(e == 1),
                                skip_group_check=True,
                            )
                        nc.vector.tensor_copy(
                            xnt[:].rearrange("p (d n) -> p d n", n=N)
                               [:, 2 * dp:2 * dp + 2, sl * 128:(sl + 1) * 128],
                            tp[:].rearrange("p (two n) -> p two n", two=2),
                        )

            def emit_qkv(h, xnt):
                """expb tile, qT/kT (fp8), v-hat for head h."""
                ones_bf, wqk, bqk, wv, bv1, pw, pb1 = consts[0]
                bt = bpool.tile([128, KT * N], BF16, tag="bias")
                nc.sync.dma_start(
                    bt[:].rearrange("p (k n) -> p k n", n=N),
                    expb_d[h].rearrange("(k p) n -> p k n", p=128),
                )
                qt = qkpool.tile([64, N], E4, tag="qt")
                ktile = qkpool.tile([64, N], E4, tag="kt")
                for c in range(2):
                    qp = mpp.tile([128, 512], F32, tag="m")
                    for dt in range(DT):
                        nc.tensor.matmul(
                            qp[:],
                            wqk[:, dt * H * 128 + h * 128:
                                dt * H * 128 + (h + 1) * 128],
                            xnt[:, dt * N + c * 512: dt * N + (c + 1) * 512],
                            start=(dt == 0),
                            stop=(not use_qk_bias and dt == DT - 1),
                        )
                    if use_qk_bias:
                        nc.tensor.matmul(
                            qp[:],
                            bqk[:, h * 128:(h + 1) * 128],
                            ones_bf[:, 0:512],
                            start=False,
                            stop=True,
                        )
                    nc.vector.tensor_copy(
                        qt[:, c * 512:(c + 1) * 512], qp[0:64, :]
                    )
                    nc.vector.tensor_copy(
                        ktile[:, c * 512:(c + 1) * 512], qp[64:128, :]
                    )
                vh = vpool.tile([128, KT * VW], BF16, tag="vh")
                nc.vector.memset(
                    vh[:].rearrange("p (s w) -> p s w", w=VW)[:, :, 256:257],
                    1.0,
                )
                for sp in range(QS // 2):  # two tok-slices per PSUM bank
                    vp = pvpp.tile([128, 512], F32, tag="pv")
                    for e in range(2):
                        sl = 2 * sp + e
                        last = (e == 1 and not use_v_bias)
                        for dt in range(DT):
                            nc.tensor.matmul(
                                vp[:, e * 256:(e + 1) * 256],
                                xnt[:, dt * N + sl * 128: dt * N + (sl + 1) * 128],
                                wv[:, dt * DH + h * 256: dt * DH + (h + 1) * 256],
                                start=(e == 0 and dt == 0),
                                stop=(last and dt == DT - 1),
                                skip_group_check=True,
                            )
                        if use_v_bias:
                            nc.tensor.matmul(
                                vp[:, e * 256:(e + 1) * 256],
                                ones_bf[:, 0:128],
                                bv1[:, h * VW: h * VW + 256],
                                start=False,
                                stop=(e == 1),
                                skip_group_check=True,
                            )
                    nc.vector.tensor_copy(
                        vh[:].rearrange("p (s w) -> p s w", w=VW)
                           [:, 2 * sp:2 * sp + 2, 0:256],
                        vp[:].rearrange("p (two v) -> p two v", two=2),
                    )
                return bt, qt, ktile, vh

            def emit_st_kt(hctx, kt):
                """Scores DR matmul + exp + expb-mult for one k-tile of the
                *next* head.  Returns the est tile."""
                bt, qt, ktile, vh = hctx
                et = epool.tile([128, N], BF16, tag="e")
                for c in range(2):
                    cs = slice(c * 512, (c + 1) * 512)
                    sp = stpp.tile([128, 512], F32, tag="st")
                    nc.tensor.matmul(
                        sp[:],
                        dup2(ktile[:, kt * 128:(kt + 1) * 128]),
                        dup2(qt[:, cs]),
                        start=True, stop=True, perf_mode=DR,
                    )
                    nc.scalar.activation(
                        et[:, cs], sp[:], AF.Exp, bias=0.0, scale=EXP_SCALE
                    )
                eng = nc.gpsimd if kt in POOL_MULT_KT else nc.vector
                eng.tensor_tensor(
                    et[:], et[:], bt[:, kt * N:(kt + 1) * N], ALU.mult
                )
                return et

            def emit_pv_sl(hctx, est, sl):
                """PV matmuls + denominator + normalized attn for one
                tok-slice.  Returns the an tile."""
                bt, qt, ktile, vh = hctx
                pv = pvpp.tile([128, VW], F32, tag="pv")
                for kt in range(KT):
                    nc.tensor.matmul(
                        pv[:],
                        est[kt][:, sl * 128:(sl + 1) * 128],
                        vh[:, kt * VW:(kt + 1) * VW],
                        start=(kt == 0),
                        stop=(kt == KT - 1),
                    )
                rc = spool.tile([128, 1], F32, tag="rc")
                nc.vector.reciprocal(rc[:], pv[:, 256:257])
                an = apool.tile([128, 256], BF16, tag="an")
                nc.scalar.mul(an[:], pv[:, 0:256], rc[:])
                return an

            def emit_fin_sl(h, ans, sl, slab):
                """Deferred: paired transpose of normalized attn into slab."""
                tp = mpp.tile([128, 256], BF16, tag="m")
                for e in range(2):
                    nc.tensor.matmul(
                        tp[:, e * 128:(e + 1) * 128],
                        ans[sl][:, e * 128:(e + 1) * 128],
                        identb[:], is_transpose=True,
                        start=(e == 0), stop=(e == 1),
                        skip_group_check=True,
                    )
                nc.vector.tensor_copy(
                    slab[:].rearrange("p (g n) -> p g n", n=N)
                        [:, 2 * h:2 * h + 2, sl * 128:(sl + 1) * 128],
                    tp[:].rearrange("p (two n) -> p two n", two=2),
                )

            def emit_window(hctx_cur, est_cur, hctx_nxt, pend, slab):
                """One head window: interleave scores/exp/mult of the next
                head, PV of the current head, and fin of the previous."""
                est_nxt = [] if hctx_nxt is not None else None
                ans = []
                for i in range(KT):
                    if hctx_nxt is not None:
                        est_nxt.append(emit_st_kt(hctx_nxt, i))
                    ans.append(emit_pv_sl(hctx_cur, est_cur, i))
                    if pend is not None:
                        emit_fin_sl(pend[0], pend[1], i, slab)
                return est_nxt, ans

            def emit_proj(b, slab, pend):
                ones_bf, wqk, bqk, wv, bv1, pw, pb1 = consts[0]
                for sl in range(QS):
                    if pend is not None:
                        emit_fin_sl(pend[0], pend[1], sl, slab)
                    pp = stpp.tile([128, DIM], F32, tag="st")
                    for dh in range(16):
                        nc.tensor.matmul(
                            pp[:],
                            slab[:, dh * N + sl * 128: dh * N + (sl + 1) * 128],
                            pw[:, dh * DIM:(dh + 1) * DIM],
                            start=(dh == 0),
                            stop=(not use_pb and dh == 15),
                        )
                    if use_pb:
                        nc.tensor.matmul(
                            pp[:], ones_bf[:, 0:128], pb1[:], start=False,
                            stop=True, skip_group_check=True,
                        )
                    yt = ypool.tile([128, DIM], F32, tag="y")
                    nc.vector.tensor_copy(yt[:], pp[:])
                    nc.sync.dma_start(y_d[b, sl * 128:(sl + 1) * 128, :], yt[:])

            # ---- software-pipelined main loop --------------------------
            LN_AT = {4: range(0, 3), 5: range(3, 6), 6: range(6, 8)}
            xnt_cur = xpool.tile([128, DT * N], BF16, tag="xnt")
            emit_ln(0, range(QS), xnt_cur)
            consts[0] = load_consts()
            hctx_cur = emit_qkv(0, xnt_cur)
            est_cur = [emit_st_kt(hctx_cur, kt) for kt in range(KT)]
            slab = slabpool.tile([128, 16 * N], BF16, tag="slab")
            xnt_next = None
            pend = None
            for b in range(BL):
                for h in range(H):
                    if h + 1 < H:
                        if b + 1 < BL and h in LN_AT:
                            if h == 4:
                                xnt_next = xpool.tile(
                                    [128, DT * N], BF16, tag="xnt", name="xnt2"
                                )
                            emit_ln(b + 1, LN_AT[h], xnt_next)
                        hctx_nxt = emit_qkv(h + 1, xnt_cur)
                    elif b + 1 < BL:
                        hctx_nxt = emit_qkv(0, xnt_next)
                    else:
                        hctx_nxt = None
                    est_nxt, ans = emit_window(
                        hctx_cur, est_cur, hctx_nxt, pend, slab
                    )
                    pend = (h, ans)
                    hctx_cur, est_cur = hctx_nxt, est_nxt
                emit_proj(b, slab, pend)
                pend = None
                xnt_cur = xnt_next

    nc.compile()
    return nc


_CACHE = {}


def _prep_host(gamma, beta, qkv_w, qkv_b, proj_w, proj_b, biases, bias_idxs):
    import ml_dtypes

    qkv_w = np.asarray(qkv_w, np.float32)
    qkv_b = np.asarray(qkv_b, np.float32)
    gamma = np.asarray(gamma, np.float32)
    beta = np.asarray(beta, np.float32)
    w = qkv_w * gamma[:, None]          # fold LN gamma
    bfold = qkv_b + beta @ qkv_w        # fold LN beta
    w3 = w.reshape(DIM, H, 384)
    b3 = bfold.reshape(H, 384)
    # q/k columns scaled x2 for fp8 range; exp scale divides it back out
    wqk = (w3[:, :, :128] * QK_PRE).reshape(DIM, H * 128)
    bqk = (b3[:, :128] * QK_PRE).reshape(1, H * 128)
    wv = w3[:, :, 128:].reshape(DIM, DH)
    bv = b3[:, 128:]                    # [H, 256]
    bv1 = np.concatenate(
        [bv, np.ones((H, 1), np.float32)], axis=1,
    ).reshape(1, H * VW)
    bias_full = np.asarray(biases, np.float32)[:, np.asarray(bias_idxs)]
    # device multiplies est[k, q] by exp(bias[q, k])^T
    expb = np.exp(bias_full.transpose(0, 2, 1))
    return {
        "wqk": wqk.astype(ml_dtypes.bfloat16),
        "wv": wv.astype(ml_dtypes.bfloat16),
        "bqk": bqk.astype(ml_dtypes.bfloat16),
        "bv1": bv1.astype(ml_dtypes.bfloat16),
        "pw": np.ascontiguousarray(np.asarray(proj_w, np.float32)).astype(ml_dtypes.bfloat16),
        "pb1": np.asarray(proj_b, np.float32).reshape(1, DIM).astype(ml_dtypes.bfloat16),
        "expb": np.ascontiguousarray(expb).astype(ml_dtypes.bfloat16),
        "identb": np.eye(128, dtype=np.float32).astype(ml_dtypes.bfloat16),
        "ones": np.ones((1, 512), ml_dtypes.bfloat16),
    }


def kernel(x, gamma, beta, qkv_w, qkv_b, proj_w, proj_b, biases, bias_idxs,
           _trace=False, _tmpdir=None):
    x = np.asarray(x, np.float32)
    shared = _prep_host(gamma, beta, qkv_w, qkv_b, proj_w, proj_b, biases,
                        bias_idxs)
    flags = (
        bool(np.any(np.asarray(shared["bqk"], np.float32))),
        bool(np.any(np.asarray(shared["bv1"], np.float32)
                    .reshape(H, VW)[:, :256])),
        bool(np.any(np.asarray(shared["pb1"], np.float32))),
    )
    if _CACHE.get("flags") != flags:
        _CACHE["nc"] = build_program(*flags)
        _CACHE["flags"] = flags
    nc = _CACHE["nc"]
    in_maps = []
    for c in range(NCORES):
        m = dict(shared)
        m["x"] = np.ascontiguousarray(x[c * BL:(c + 1) * BL])
        in_maps.append(m)
    res = run_bass_kernel_spmd(
        nc, in_maps, list(range(NCORES)), trace=_trace, tmpdir=_tmpdir,
    )
    _CACHE["last"] = res
    out = np.concatenate([res.results[c]["y"] for c in range(NCORES)], axis=0)
    return out.astype(np.float32)


# revision 7
# speedup vs baseline: 1.2421x; 1.0031x over previous
"""Trainium2 Bass kernel for nn_Attention_51376398794919.

Dense transformer block: LayerNorm -> QKV -> attention with relative-position
bias -> proj.  Data-parallel over batch across 8 NeuronCores (4 batches/core).

Device-side layout strategy (per core):
  - LN in natural layout [tok, d]; 1/sigma via a fast-inverse-sqrt bit trick
    + 2 Newton steps on the DVE (keeps the ACT engine on a single activation
    table set: only Exp and Copy are ever used -> one table load total);
    xn transposed to xnT [d, tok] via paired PE transposes (stored bf16).
  - qkT (q/k head-transposed, [d_head, tok]) computed from xnT and quantized
    to fp8e4m3 (scaled x2 on the host); scores run as a single fp8 DoubleRow
    matmul per (k-tile, chunk) with a stride-0 pair dim (reads each operand
    twice -> 2x result, folded into the exp scale 1/64).
  - Relative-position bias applied multiplicatively: host precomputes
    expb = exp(bias^T) bf16; est = exp(scores/64) (ACT) then est *= expb
    (DVE for k-tiles 0-4, GPSIMD for 5-7).  No bias matmuls on the PE.
  - PV: out[q, d] = expST.T @ [v | ones]; the ones column yields the softmax
    denominator; normalization on ACT (Copy with per-partition 1/den scale).
  - v-hat generated two token-slices per PSUM bank (shared-bank accumulation
    groups) so each PSUM->SBUF copy moves 512 columns; attn-out transposes
    are paired the same way.
Software pipelining: each head window interleaves, at k-tile granularity,
scores+exp+mult of head h+1, PV matmuls + normalization of head h, and the
deferred attn-out transposes/copies of head h-1 (which wait on the ACT
chain).  LN of batch b+1 is emitted during heads 4-6 of batch b; qkv/scores
of (head 0, b+1) run before pv(7, b); fin(7) interleaves into proj(b).
"""

import sys

import numpy as np

sys.path.insert(0, "/opt/trn_rl_repo")

import concourse.bacc as bacc
import concourse.mybir as mybir
import concourse.tile as tile
from concourse.bass_utils import run_bass_kernel_spmd

# Problem constants
B, N, DIM = 32, 1024, 512
H, KD, D = 8, 64, 256
DH = D * H  # 2048
SCALE = KD ** -0.5
NCORES = 8
BL = B // NCORES  # 4 batches per core

F32 = mybir.dt.float32
I32 = mybir.dt.int32
BF16 = mybir.dt.bfloat16
E4 = mybir.dt.float8e4
AF = mybir.ActivationFunctionType
ALU = mybir.AluOpType
DR = mybir.MatmulPerfMode.DoubleRow

KT = N // 128    # 8 k-tiles
QS = N // 128    # 8 q-slices
DT = DIM // 128  # 4 d-tiles
VW = 257         # v-hat width: 256 v + 1 ones
QK_PRE = 2.0     # host pre-scale on q and k before fp8 quantization
# scores psum = 2 (stride-0 pair) * (2q)*(2k) = 8*qk ; exp scale recovers /8
EXP_SCALE = 1.0 / (2 * QK_PRE * QK_PRE) * SCALE
POOL_MULT_KT = (5, 6, 7)  # est *= expb k-tiles run on GPSIMD


def dup2(ap):
    """Insert a stride-0 count-2 dim after the partition dim (DoubleRow
    pair that reads the same block twice -> result is 2x)."""
    new = ap.copy()
    new.ap = [ap.ap[0]] + [[0, 2]] + ap.ap[1:]
    return new


def build_program(use_qk_bias=False, use_v_bias=False, use_pb=False):
    nc = bacc.Bacc("TRN2", target_bir_lowering=False, debug=True)

    x_d = nc.declare_dram_parameter("x", [BL, N, DIM], F32, isOutput=False)
    wqk_d = nc.declare_dram_parameter("wqk", [DIM, H * 128], BF16, isOutput=False)
    wv_d = nc.declare_dram_parameter("wv", [DIM, DH], BF16, isOutput=False)
    bqk_d = nc.declare_dram_parameter("bqk", [1, H * 128], BF16, isOutput=False)
    bv1_d = nc.declare_dram_parameter("bv1", [1, H * VW], BF16, isOutput=False)
    pw_d = nc.declare_dram_parameter("pw", [DH, DIM], BF16, isOutput=False)
    pb1_d = nc.declare_dram_parameter("pb1", [1, DIM], BF16, isOutput=False)
    expb_d = nc.declare_dram_parameter("expb", [H, N, N], BF16, isOutput=False)
    identb_d = nc.declare_dram_parameter("identb", [128, 128], BF16, isOutput=False)
    ones_d = nc.declare_dram_parameter("ones", [1, 512], BF16, isOutput=False)
    y_d = nc.declare_dram_parameter("y", [BL, N, DIM], F32, isOutput=True)

    with tile.TileContext(nc) as tc:
        with (
            tc.tile_pool(name="consts", bufs=1) as cpool,
            tc.tile_pool(name="xnt", bufs=2) as xpool,
            tc.tile_pool(name="slab", bufs=1) as slabpool,
            tc.tile_pool(name="yout", bufs=3) as ypool,
            tc.tile_pool(name="lnx", bufs=8) as lxpool,
            tc.tile_pool(name="ln", bufs=3) as lpool,
            tc.tile_pool(name="stats", bufs=16) as spool,
            tc.tile_pool(name="bias", bufs=2) as bpool,
            tc.tile_pool(name="qk", bufs=3) as qkpool,
            tc.tile_pool(name="vhat", bufs=2) as vpool,
            tc.tile_pool(name="expst", bufs=16) as epool,
            tc.tile_pool(name="attn", bufs=10) as apool,
            tc.tile_pool(name="stp", bufs=2, space="PSUM") as stpp,
            tc.tile_pool(name="pvp", bufs=4, space="PSUM") as pvpp,
            tc.tile_pool(name="miscp", bufs=2, space="PSUM") as mpp,
        ):
            # ---- constants; identb first so LN transposes can start early
            identb = cpool.tile([128, 128], BF16)
            nc.sync.dma_start(identb[:], identb_d[:])
            eps_t = cpool.tile([128, 1], F32)
            nc.vector.memset(eps_t[:], 1e-5)

            def load_consts():
                if use_qk_bias or use_v_bias or use_pb:
                    ones_bf = cpool.tile([1, 512], BF16)
                    nc.sync.dma_start(ones_bf[:], ones_d[:])
                else:
                    ones_bf = None
                wqk = cpool.tile([128, DT * H * 128], BF16)
                for dt in range(DT):
                    for hh in range(2):
                        nc.sync.dma_start(
                            wqk[:, dt * H * 128 + hh * 512:
                                dt * H * 128 + (hh + 1) * 512],
                            wqk_d[dt * 128:(dt + 1) * 128,
                                  hh * 512:(hh + 1) * 512],
                        )
                if use_qk_bias:
                    bqk = cpool.tile([1, H * 128], BF16)
                    nc.sync.dma_start(bqk[:], bqk_d[:])
                else:
                    bqk = None
                wv = cpool.tile([128, DT * DH], BF16)
                for dt in range(DT):
                    for hh in range(2):
                        nc.sync.dma_start(
                            wv[:, dt * DH + hh * (DH // 2):
                               dt * DH + (hh + 1) * (DH // 2)],
                            wv_d[dt * 128:(dt + 1) * 128,
                                 hh * (DH // 2):(hh + 1) * (DH // 2)],
                        )
                if use_v_bias:
                    bv1 = cpool.tile([1, H * VW], BF16)
                    nc.sync.dma_start(bv1[:], bv1_d[:])
                else:
                    bv1 = None
                pw = cpool.tile([128, 16 * DIM], BF16)
                for dh in range(16):
                    nc.sync.dma_start(
                        pw[:, dh * DIM:(dh + 1) * DIM],
                        pw_d[dh * 128:(dh + 1) * 128, :],
                    )
                if use_pb:
                    pb1 = cpool.tile([1, DIM], BF16)
                    nc.sync.dma_start(pb1[:], pb1_d[:])
                else:
                    pb1 = None
                return ones_bf, wqk, bqk, wv, bv1, pw, pb1

            consts = [None]

            def emit_ln(b, sls, xnt):
                """LayerNorm slices `sls` of batch b into xnT [d, tok].
                1/sigma via bit-trick + 2 Newton steps, all on DVE."""
                L = len(sls)
                mvg = spool.tile([128, 2 * L], F32, tag="mvg")
                xts = []
                for j, sl in enumerate(sls):
                    xt = lxpool.tile([128, DIM], F32, tag="x")
                    nc.sync.dma_start(xt[:], x_d[b, sl * 128:(sl + 1) * 128, :])
                    xts.append(xt)
                    st6 = spool.tile([128, 6], F32, tag="st6")
                    nc.vector.bn_stats(st6[:], xt[:])
                    nc.vector.bn_aggr(mvg[:, 2 * j:2 * j + 2], st6[:])
                var_ap = mvg[:].rearrange("p (l two) -> p l two", two=2)[:, :, 1]
                ve = spool.tile([128, L], F32, tag="ve")
                nc.vector.tensor_scalar(ve[:], var_ap, eps_t[:], None, ALU.add)
                ti = spool.tile([128, L], I32, tag="ti")
                nc.vector.tensor_scalar(
                    ti[:], ve[:].bitcast(I32), 1, 0xFFFFFFFF,
                    ALU.logical_shift_right, ALU.bitwise_xor,
                )
                yi = spool.tile([128, L], I32, tag="yi")
                nc.vector.tensor_scalar(yi[:], ti[:], 0x5F3759E0, None, ALU.add)
                y = yi[:].bitcast(F32)
                aa = spool.tile([128, L], F32, tag="aa")
                cc = spool.tile([128, L], F32, tag="cc")
                for _ in range(2):
                    nc.vector.tensor_tensor(aa[:], y, y, ALU.mult)
                    nc.vector.tensor_tensor(aa[:], aa[:], ve[:], ALU.mult)
                    nc.vector.tensor_scalar(
                        cc[:], aa[:], -0.5, 1.5, ALU.mult, ALU.add
                    )
                    nc.vector.tensor_tensor(y, y, cc[:], ALU.mult)
                for j, sl in enumerate(sls):
                    rs = yi[:, j:j + 1].bitcast(F32)
                    nm = spool.tile([128, 1], F32, tag="nm")
                    nc.vector.tensor_scalar(
                        nm[:], mvg[:, 2 * j:2 * j + 1], rs, -1.0,
                        ALU.mult, ALU.mult
                    )
                    xn = lpool.tile([128, DIM], BF16, tag="xn")
                    nc.vector.tensor_scalar(
                        xn[:], xts[j][:], rs, nm[:], ALU.mult, ALU.add
                    )
                    for dp in range(2):  # pairs of d-tiles
                        tp = mpp.tile([128, 256], BF16, tag="m")
                        for e in range(2):
                            nc.tensor.matmul(
                                tp[:, e * 128:(e + 1) * 128],
                                xn[:, (2 * dp + e) * 128:(2 * dp + e + 1) * 128],
                                identb[:], is_transpose=True,
                                start=(e == 0), stop=(e == 1),
                                skip_group_check=True,
                            )
                        nc.vector.tensor_copy(
                            xnt[:].rearrange("p (d n) -> p d n", n=N)
                               [:, 2 * dp:2 * dp + 2, sl * 128:(sl + 1) * 128],
                            tp[:].rearrange("p (two n) -> p two n", two=2),
                        )

            def emit_qkv(h, xnt):
                """expb tile, qT/kT (fp8), v-hat for head h."""
                ones_bf, wqk, bqk, wv, bv1, pw, pb1 = consts[0]
                bt = bpool.tile([128, KT * N], BF16, tag="bias")
                nc.sync.dma_start(
                    bt[:].rearrange("p (k n) -> p k n", n=N),
                    expb_d[h].rearrange("(k p) n -> p k n", p=128),
                )
                qt = qkpool.tile([64, N], E4, tag="qt")
                ktile = qkpool.tile([64, N], E4, tag="kt")
                for c in range(2):
                    qp = mpp.tile([128, 512], F32, tag="m")
                    for dt in range(DT):
                        nc.tensor.matmul(
                            qp[:],
                            wqk[:, dt * H * 128 + h * 128:
                                dt * H * 128 + (h + 1) * 128],
                            xnt[:, dt * N + c * 512: dt * N + (c + 1) * 512],
                            start=(dt == 0),
                            stop=(not use_qk_bias and dt == DT - 1),
                        )
                    if use_qk_bias:
                        nc.tensor.matmul(
                            qp[:],
                            bqk[:, h * 128:(h + 1) * 128],
                            ones_bf[:, 0:512],
                            start=False,
                            stop=True,
                        )
                    nc.vector.tensor_copy(
                        qt[:, c * 512:(c + 1) * 512], qp[0:64, :]
                    )
                    nc.vector.tensor_copy(
                        ktile[:, c * 512:(c + 1) * 512], qp[64:128, :]
                    )
                vh = vpool.tile([128, KT * VW], BF16, tag="vh")
                nc.vector.memset(
                    vh[:].rearrange("p (s w) -> p s w", w=VW)[:, :, 256:257],
                    1.0,
                )
                for sp in range(QS // 2):  # two tok-slices per PSUM bank
                    vp = pvpp.tile([128, 512], F32, tag="pv")
                    for e in range(2):
                        sl = 2 * sp + e
                        last = (e == 1 and not use_v_bias)
                        for dt in range(DT):
                            nc.tensor.matmul(
                                vp[:, e * 256:(e + 1) * 256],
                                xnt[:, dt * N + sl * 128: dt * N + (sl + 1) * 128],
                                wv[:, dt * DH + h * 256: dt * DH + (h + 1) * 256],
                                start=(e == 0 and dt == 0),
                                stop=(last and dt == DT - 1),
                                skip_group_check=True,
                            )
                        if use_v_bias:
                            nc.tensor.matmul(
                                vp[:, e * 256:(e + 1) * 256],
                                ones_bf[:, 0:128],
                                bv1[:, h * VW: h * VW + 256],
                                start=False,
                                stop=(e == 1),
                                skip_group_check=True,
                            )
                    nc.vector.tensor_copy(
                        vh[:].rearrange("p (s w) -> p s w", w=VW)
                           [:, 2 * sp:2 * sp + 2, 0:256],
                        vp[:].rearrange("p (two v) -> p two v", two=2),
                    )
                return bt, qt, ktile, vh

            def emit_st_kt(hctx, kt):
                """Scores DR matmul + exp + expb-mult for one k-tile of the
                *next* head.  Returns the est tile."""
                bt, qt, ktile, vh = hctx
                et = epool.tile([128, N], BF16, tag="e")
                for c in range(2):
                    cs = slice(c * 512, (c + 1) * 512)
                    sp = stpp.tile([128, 512], F32, tag="st")
                    nc.tensor.matmul(
                        sp[:],
                        dup2(ktile[:, kt * 128:(kt + 1) * 128]),
                        dup2(qt[:, cs]),
                        start=True, stop=True, perf_mode=DR,
                    )
                    nc.scalar.activation(
                        et[:, cs], sp[:], AF.Exp, bias=0.0, scale=EXP_SCALE
                    )
                eng = nc.gpsimd if kt in POOL_MULT_KT else nc.vector
                eng.tensor_tensor(
                    et[:], et[:], bt[:, kt * N:(kt + 1) * N], ALU.mult
                )
                return et

            def emit_pv_sl(hctx, est, sl):
                """PV matmuls + denominator + normalized attn for one
                tok-slice.  Returns the an tile."""
                bt, qt, ktile, vh = hctx
                pv = pvpp.tile([128, VW], F32, tag="pv")
                for kt in range(KT):
                    nc.tensor.matmul(
                        pv[:],
                        est[kt][:, sl * 128:(sl + 1) * 128],
                        vh[:, kt * VW:(kt + 1) * VW],
                        start=(kt == 0),
                        stop=(kt == KT - 1),
                    )
                rc = spool.tile([128, 1], F32, tag="rc")
                nc.vector.reciprocal(rc[:], pv[:, 256:257])
                an = apool.tile([128, 256], BF16, tag="an")
                nc.scalar.mul(an[:], pv[:, 0:256], rc[:])
                return an

            def emit_fin_sl(h, ans, sl, slab):
                """Deferred: paired transpose of normalized attn into slab."""
                tp = mpp.tile([128, 256], BF16, tag="m")
                for e in range(2):
                    nc.tensor.matmul(
                        tp[:, e * 128:(e + 1) * 128],
                        ans[sl][:, e * 128:(e + 1) * 128],
                        identb[:], is_transpose=True,
                        start=(e == 0), stop=(e == 1),
                        skip_group_check=True,
                    )
                nc.vector.tensor_copy(
                    slab[:].rearrange("p (g n) -> p g n", n=N)
                        [:, 2 * h:2 * h + 2, sl * 128:(sl + 1) * 128],
                    tp[:].rearrange("p (two n) -> p two n", two=2),
                )

            def emit_window(hctx_cur, est_cur, hctx_nxt, pend, slab):
                """One head window: interleave scores/exp/mult of the next
                head, PV of the current head, and fin of the previous."""
                est_nxt = [] if hctx_nxt is not None else None
                ans = []
                for i in range(KT):
                    if hctx_nxt is not None:
                        est_nxt.append(emit_st_kt(hctx_nxt, i))
                    ans.append(emit_pv_sl(hctx_cur, est_cur, i))
                    if pend is not None:
                        emit_fin_sl(pend[0], pend[1], i, slab)
                return est_nxt, ans

            def emit_proj(b, slab, pend):
                ones_bf, wqk, bqk, wv, bv1, pw, pb1 = consts[0]
                for sl in range(QS):
                    if pend is not None:
                        emit_fin_sl(pend[0], pend[1], sl, slab)
                    pp = stpp.tile([128, DIM], F32, tag="st")
                    for dh in range(16):
                        nc.tensor.matmul(
                            pp[:],
                            slab[:, dh * N + sl * 128: dh * N + (sl + 1) * 128],
                            pw[:, dh * DIM:(dh + 1) * DIM],
                            start=(dh == 0),
                            stop=(not use_pb and dh == 15),
                        )
                    if use_pb:
                        nc.tensor.matmul(
                            pp[:], ones_bf[:, 0:128], pb1[:], start=False,
                            stop=True, skip_group_check=True,
                        )
                    yt = ypool.tile([128, DIM], F32, tag="y")
                    nc.vector.tensor_copy(yt[:], pp[:])
                    nc.sync.dma_start(y_d[b, sl * 128:(sl + 1) * 128, :], yt[:])

            # ---- software-pipelined main loop --------------------------
            LN_AT = {4: range(0, 3), 5: range(3, 6), 6: range(6, 8)}
            xnt_cur = xpool.tile([128, DT * N], BF16, tag="xnt")
            emit_ln(0, range(QS), xnt_cur)
            consts[0] = load_consts()
            hctx_cur = emit_qkv(0, xnt_cur)
            est_cur = [emit_st_kt(hctx_cur, kt) for kt in range(KT)]
            slab = slabpool.tile([128, 16 * N], BF16, tag="slab")
            xnt_next = None
            pend = None
            for b in range(BL):
                for h in range(H):
                    if h + 1 < H:
                        if b + 1 < BL and h in LN_AT:
                            if h == 4:
                                xnt_next = xpool.tile(
                                    [128, DT * N], BF16, tag="xnt", name="xnt2"
                                )
                            emit_ln(b + 1, LN_AT[h], xnt_next)
                        hctx_nxt = emit_qkv(h + 1, xnt_cur)
                    elif b + 1 < BL:
                        hctx_nxt = emit_qkv(0, xnt_next)
                    else:
                        hctx_nxt = None
                    est_nxt, ans = emit_window(
                        hctx_cur, est_cur, hctx_nxt, pend, slab
                    )
                    pend = (h, ans)
                    hctx_cur, est_cur = hctx_nxt, est_nxt
                emit_proj(b, slab, pend)
                pend = None
                xnt_cur = xnt_next

    nc.compile()
    return nc


_CACHE = {}


def _prep_host(gamma, beta, qkv_w, qkv_b, proj_w, proj_b, biases, bias_idxs):
    import ml_dtypes

    qkv_w = np.asarray(qkv_w, np.float32)
    qkv_b = np.asarray(qkv_b, np.float32)
    gamma = np.asarray(gamma, np.float32)
    beta = np.asarray(beta, np.float32)
    w = qkv_w * gamma[:, None]          # fold LN gamma
    bfold = qkv_b + beta @ qkv_w        # fold LN beta
    w3 = w.reshape(DIM, H, 384)
    b3 = bfold.reshape(H, 384)
    # q/k columns scaled x2 for fp8 range; exp scale divides it back out
    wqk = (w3[:, :, :128] * QK_PRE).reshape(DIM, H * 128)
    bqk = (b3[:, :128] * QK_PRE).reshape(1, H * 128)
    wv = w3[:, :, 128:].reshape(DIM, DH)
    bv = b3[:, 128:]                    # [H, 256]
    bv1 = np.concatenate(
        [bv, np.ones((H, 1), np.float32)], axis=1,
    ).reshape(1, H * VW)
    bias_full = np.asarray(biases, np.float32)[:, np.asarray(bias_idxs)]
    # device multiplies est[k, q] by exp(bias[q, k])^T
    expb = np.exp(bias_full.transpose(0, 2, 1))
    return {
        "wqk": wqk.astype(ml_dtypes.bfloat16),
        "wv": wv.astype(ml_dtypes.bfloat16),
        "bqk": bqk.astype(ml_dtypes.bfloat16),
        "bv1": bv1.astype(ml_dtypes.bfloat16),
        "pw": np.ascontiguousarray(np.asarray(proj_w, np.float32)).astype(ml_dtypes.bfloat16),
        "pb1": np.asarray(proj_b, np.float32).reshape(1, DIM).astype(ml_dtypes.bfloat16),
        "expb": np.ascontiguousarray(expb).astype(ml_dtypes.bfloat16),
        "identb": np.eye(128, dtype=np.float32).astype(ml_dtypes.bfloat16),
        "ones": np.ones((1, 512), ml_dtypes.bfloat16),
    }


def kernel(x, gamma, beta, qkv_w, qkv_b, proj_w, proj_b, biases, bias_idxs,
           _trace=False, _tmpdir=None):
    x = np.asarray(x, np.float32)
    shared = _prep_host(gamma, beta, qkv_w, qkv_b, proj_w, proj_b, biases,
                        bias_idxs)
    flags = (
        bool(np.any(np.asarray(shared["bqk"], np.float32))),
        bool(np.any(np.asarray(shared["bv1"], np.float32)
                    .reshape(H, VW)[:, :256])),
        bool(np.any(np.asarray(shared["pb1"], np.float32))),
    )
    if _CACHE.get("flags") != flags:
        _CACHE["nc"] = build_program(*flags)
        _CACHE["flags"] = flags
    nc = _CACHE["nc"]
    in_maps = []
    for c in range(NCORES):
        m = dict(shared)
        m["x"] = np.ascontiguousarray(x[c * BL:(c + 1) * BL])
        in_maps.append(m)
    res = run_bass_kernel_spmd(
        nc, in_maps, list(range(NCORES)), trace=_trace, tmpdir=_tmpdir,
    )
    _CACHE["last"] = res
    out = np.concatenate([res.results[c]["y"] for c in range(NCORES)], axis=0)
    return out.astype(np.float32)
